# revision 2
# baseline (speedup 1.0000x reference)
"""Multi-head causal self-attention on 8 trn2 NeuronCores, v2.

Problem: x[2,2048,1024], 16 heads x 64 dim, causal softmax attention,
QKV/O projections with biases.

Sharding: core c handles batch b=c//4, head group g=c%4 (heads 4g..4g+3).
Each core computes its 4 heads' attention plus the partial O-projection;
the host sums the 4 partials per batch and adds bo.

v2 design (vs baseline):
- all matmuls bf16 (same PE rate as f32r, half the DMA/SBUF)
- causal column-trimming: diagonal k-tiles only compute/exp/accumulate
  q-columns >= 128*r (the valid trapezoid), saving ~15% of S/PV/exp
- additive -30000 triangle mask applied to the score PSUM *before* exp
  (one static [128,2,128] mask tile), so exp emits exact zeros and no
  post-exp mask multiply is needed
- O-projection packs 2 heads per matmul (128-row contraction): 64
  matmuls instead of 128
- denominator: ones-column in V (row 64 of cps), reciprocal on DVE,
  partition_broadcast on Pool (no DRAM bounce)
- V-projection and O-projection matmuls are interleaved into the
  attention k-loops as PE filler; PSUM rotates across 8 banks
"""
import os
import sys

if os.path.isdir("/opt/trn_rl_repo"):
    sys.path.insert(0, "/opt/trn_rl_repo")

import numpy as np
import ml_dtypes

import concourse.bass as bass  # noqa: F401
import concourse.tile as tile
from concourse import bacc
from concourse import mybir

F32 = mybir.dt.float32
F32R = mybir.dt.float32r
BF16 = mybir.dt.bfloat16
AF = mybir.ActivationFunctionType
ADD = mybir.AluOpType.add
MULT = mybir.AluOpType.mult

T = 2048          # sequence length
C = 1024          # model dim
HG = 4            # heads per core
HD = 64           # head dim
DG = HG * HD      # 256, projected dims per core
NF = C // 128     # 8 feature chunks
NT = T // 128     # 16 token tiles
NQ = T // 512     # 4 q-chunks
SCALE = 0.125     # 1/sqrt(64)
LAG = 4           # exp -> PV pipeline lag (in k-tiles)

BFNP = ml_dtypes.bfloat16


def build_kernel():
    nc = bacc.Bacc("TRN2")
    xT_d = nc.dram_tensor("xT", [C, T], BF16, kind="ExternalInput").ap()
    wq_d = nc.dram_tensor("wq", [128, NF * DG], BF16, kind="ExternalInput").ap()
    wk_d = nc.dram_tensor("wk", [128, NF * DG], BF16, kind="ExternalInput").ap()
    wv_d = nc.dram_tensor("wv", [128, NF * DG], BF16, kind="ExternalInput").ap()
    wo_d = nc.dram_tensor("wo", [2, 128, C], BF16, kind="ExternalInput").ap()
    bq_d = nc.dram_tensor("bq", [128, 2], F32, kind="ExternalInput").ap()
    bk_d = nc.dram_tensor("bk", [128, 2], F32, kind="ExternalInput").ap()
    bv_d = nc.dram_tensor("bv", [128, DG], F32, kind="ExternalInput").ap()
    out_d = nc.dram_tensor("out", [T, C], F32, kind="ExternalOutput").ap()

    with tile.TileContext(nc) as tc:
        with tc.tile_pool(name="persist", bufs=1) as pp:
            qt = pp.tile([128, 2, T], BF16, name="qt")    # [d'128, pair, t]
            kt = pp.tile([128, 2, T], BF16, name="kt")
            vsb = pp.tile([128, NT, HG, HD + 1], BF16, name="vsb")  # [V|1]
            ctxT = [pp.tile([128, T], BF16, name=f"ctxT{p}") for p in range(2)]
            wo_sb = pp.tile([128, 2, C], BF16, name="wo_sb")
            bq_sb = pp.tile([128, 2], F32, name="bq_sb")
            bk_sb = pp.tile([128, 2], F32, name="bk_sb")
            bv_sb = pp.tile([128, DG], F32, name="bv_sb")
            mask2 = pp.tile([128, 2, 128], BF16, name="mask2")
            ones = pp.tile([128, 64], BF16, name="ones")
            nc.gpsimd.memset(ones[:], 1.0)

            # ones column of V_ext
            nc.gpsimd.memset(vsb[:, :, :, HD:HD + 1], 1.0)
            # multiplicative triangle mask: 1 where col >= partition, else 0
            nc.gpsimd.memset(mask2[:], 1.0)
            for j in range(2):
                nc.gpsimd.affine_select(
                    out=mask2[:, j, :],
                    in_=mask2[:, j, :],
                    compare_op=mybir.AluOpType.is_ge,
                    fill=0.0,
                    base=0,
                    pattern=[[1, 128]],
                    channel_multiplier=-1,
                )

            with tc.tile_pool(name="xtp", bufs=1) as xtp, \
                 tc.tile_pool(name="wp", bufs=2) as wp:
                xt = xtp.tile([128, NF, T], BF16, name="xt")

                w_srcs = {"q": wq_d, "k": wk_d, "v": wv_d}
                w_tiles = {}

                def load_w(which, eng, split=False):
                    w_tiles[which] = wp.tile([128, NF, DG], BF16,
                                             name=f"w{which}", tag="w")
                    src_v = w_srcs[which].rearrange("p (f d) -> p f d", f=NF)
                    if split:
                        for f in range(NF):
                            eng.dma_start(w_tiles[which][:, f, :],
                                          src_v[:, f, :])
                    else:
                        eng.dma_start(w_tiles[which][:], src_v)

                load_w("k", nc.scalar, split=True)
                load_w("q", nc.gpsimd)
                load_w("v", nc.gpsimd)
                # first chunk in 512-col pieces so the first matmul can
                # start as soon as 0.125MB lands
                for t4 in range(NQ):
                    nc.sync.dma_start(
                        xt[:, 0, 512 * t4:512 * (t4 + 1)],
                        xT_d[0:128, 512 * t4:512 * (t4 + 1)])
                for f in range(1, NF):
                    nc.sync.dma_start(xt[:, f, :], xT_d[128 * f:128 * (f + 1), :])
                # late, off the critical path: biases + wo (sync queue,
                # after the xt chunks that gate the first matmuls)
                nc.sync.dma_start(bq_sb[:], bq_d)
                nc.sync.dma_start(bk_sb[:], bk_d)
                nc.sync.dma_start(bv_sb[:], bv_d)
                for p in range(2):
                    nc.sync.dma_start(wo_sb[:, p, :], wo_d[p])

                # ---- QK projections: psum [128, 2048] per (dst, dc) ----
                _sid_p, _ = nc.enter_named_scope("proj", False)
                with tc.tile_pool(name="pjp", bufs=2, space="PSUM") as pjp:
                    for dc in range(2):
                        for dst, wkey, b_sb in ((kt, "k", bk_sb),
                                                (qt, "q", bq_sb)):
                            w_sb = w_tiles[wkey]
                            ps = pjp.tile([128, T], F32, name="pjqk", tag="pj")
                            for f in range(NF):
                                lhsT = w_sb[:, f, 128 * dc:128 * (dc + 1)]
                                for t4 in range(NQ):
                                    nc.tensor.matmul(
                                        ps[:, 512 * t4:512 * (t4 + 1)],
                                        lhsT,
                                        xt[:, f, 512 * t4:512 * (t4 + 1)],
                                        start=(f == 0), stop=(f == NF - 1),
                                    )
                            if wkey == "k":
                                # ACT is idle during proj: bias-add + copy
                                nc.scalar.activation(
                                    dst[:, dc, :], ps[:], AF.Identity,
                                    bias=b_sb[:, dc:dc + 1])
                            else:
                                for t4 in range(NQ):
                                    nc.vector.tensor_scalar_add(
                                        dst[:, dc, 512 * t4:512 * (t4 + 1)],
                                        ps[:, 512 * t4:512 * (t4 + 1)],
                                        b_sb[:, dc:dc + 1])
                nc.leave_named_scope("proj", _sid_p, False)

                # ---- phase B + V-proj/O-proj as PE filler ----
                wv_sb = w_tiles["v"]
                with tc.tile_pool(name="pp2", bufs=8) as pbuf, \
                     tc.tile_pool(name="outp", bufs=6) as outp, \
                     tc.tile_pool(name="dnp", bufs=4) as dnp, \
                     tc.tile_pool(name="bcp", bufs=2) as bcp, \
                     tc.tile_pool(name="sps", bufs=2, space="PSUM") as sps, \
                     tc.tile_pool(name="cps", bufs=2, space="PSUM") as cps, \
                     tc.tile_pool(name="vop", bufs=2, space="PSUM") as vop:

                    filler = []

                    def v_group(t):
                        def emit():
                            ps = vop.tile([128, DG], F32, name="vps", tag="vo")
                            for f in range(NF):
                                nc.tensor.matmul(
                                    ps[:],
                                    xt[:, f, 128 * t:128 * (t + 1)],
                                    wv_sb[:, f, :],
                                    start=(f == 0), stop=(f == NF - 1),
                                )
                            nc.vector.tensor_tensor(
                                vsb[:, t, :, 0:HD],
                                ps[:].rearrange("p (h d) -> p h d", h=HG),
                                bv_sb[:].rearrange("p (h d) -> p h d", h=HG),
                                ADD)
                        return emit

                    def o_group(t0, c2, alt=[0], drain=False):
                        def emit():
                            o_ps = vop.tile([128, 512], F32, name="ops",
                                            tag="vo")
                            for p in range(2):
                                nc.tensor.matmul(
                                    o_ps[:],
                                    ctxT[p][:, t0:t0 + 128],
                                    wo_sb[:, p, 512 * c2:512 * (c2 + 1)],
                                    start=(p == 0), stop=(p == 1),
                                )
                            o_sb = outp.tile([128, 512], F32, name="osb",
                                             tag="osb")
                            a = alt[0] = alt[0] + 1
                            if drain:
                                # split copy DVE/ACT + DMAs on two queues so
                                # the end-of-kernel drain parallelizes
                                nc.vector.tensor_copy(o_sb[:, 0:256],
                                                      o_ps[:, 0:256])
                                nc.scalar.activation(o_sb[:, 256:512],
                                                     o_ps[:, 256:512],
                                                     AF.Identity)
                                nc.sync.dma_start(
                                    out_d[t0:t0 + 128,
                                          512 * c2:512 * c2 + 256],
                                    o_sb[:, 0:256])
                                nc.gpsimd.dma_start(
                                    out_d[t0:t0 + 128,
                                          512 * c2 + 256:512 * (c2 + 1)],
                                    o_sb[:, 256:512])
                            else:
                                nc.vector.tensor_copy(o_sb[:], o_ps[:])
                                (nc.sync, nc.gpsimd, nc.scalar)[a % 3].dma_start(
                                    out_d[t0:t0 + 128,
                                          512 * c2:512 * (c2 + 1)],
                                    o_sb[:])
                        return emit

                    for t in range(NT):
                        filler.append(v_group(t))

                    def pop_filler(n=1):
                        for _ in range(n):
                            if filler:
                                filler.pop(0)()

                    for qc in range(NQ):
                        _sid_a, _ = nc.enter_named_scope(f"attn{qc}", False)
                        nkt = 4 * qc + 4
                        for pair in range(2):
                            heads = (2 * pair, 2 * pair + 1)
                            cps_t = {h: cps.tile([HD + 1, 512], F32,
                                                 name=f"cps{h}", tag="cps")
                                     for h in heads}
                            pts = {}
                            # 2-ki batches: 4 S matmuls, 2 exps, then 4 PV
                            # matmuls — longer same-shape PE runs
                            for kb in range(0, nkt + LAG, 2):
                                if kb < nkt:
                                    sts = {}
                                    for ki in (kb, kb + 1):
                                        r = ki - 4 * qc
                                        lo = 128 * r if r > 0 else 0
                                        sts[ki] = (lo, sps.tile(
                                            [128, 2, 512], F32,
                                            name="s_ps", tag="s"))
                                        for j in range(2):
                                            nc.tensor.matmul(
                                                sts[ki][1][:, j, lo:512],
                                                kt[64 * j:64 * j + 64, pair,
                                                   128 * ki:128 * (ki + 1)],
                                                qt[64 * j:64 * j + 64, pair,
                                                   512 * qc + lo:
                                                   512 * (qc + 1)],
                                                start=True, stop=True)
                                    for ki in (kb, kb + 1):
                                        lo, s_ps = sts[ki]
                                        r = ki - 4 * qc
                                        p_t = pbuf.tile([128, 2, 512], BF16,
                                                        name="p", tag="p")
                                        if lo > 0:
                                            # PV runs full-width; zero the
                                            # left-of-trapezoid region
                                            nc.gpsimd.memset(
                                                p_t[:, :, 0:lo], 0.0)
                                        nc.scalar.activation(
                                            p_t[:, :, lo:512],
                                            s_ps[:, :, lo:512],
                                            AF.Exp, scale=SCALE)
                                        if r >= 0:
                                            # zero the above-diag triangle
                                            # (p is SBUF bf16 -> Pool-able)
                                            nc.gpsimd.tensor_tensor(
                                                p_t[:, :, lo:lo + 128],
                                                p_t[:, :, lo:lo + 128],
                                                mask2[:], MULT)
                                        pts[ki] = p_t
                                pop_filler()
                                if kb >= LAG:
                                    for k in (kb - LAG, kb - LAG + 1):
                                        pk = pts.pop(k)
                                        for j, h in enumerate(heads):
                                            nc.tensor.matmul(
                                                cps_t[h][:],
                                                vsb[:, k, h, :],
                                                pk[:, j, :],
                                                start=(k == 0),
                                                stop=(k == nkt - 1),
                                            )
                            # denominators + normalize into ctxT[pair]:
                            # bf16-cast the cps tile (for its denom row),
                            # broadcast the row across 64 partitions with a
                            # 1-row bf16 matmul, reciprocal (partition-0
                            # aligned), then scale ctx
                            for j, h in enumerate(heads):
                                dnb = dnp.tile([65, 512], BF16, name="dnb",
                                               tag="dnb")
                                nc.vector.tensor_copy(dnb[:], cps_t[h][:])
                                bc_ps = vop.tile([64, 512], F32, name="bc",
                                                 tag="vo")
                                nc.tensor.matmul(
                                    bc_ps[:],
                                    ones[64:65, :],
                                    dnb[64:65, :],
                                    start=True, stop=True)
                                bcr = dnp.tile([64, 512], F32, name="bcr",
                                               tag="bcr")
                                nc.vector.reciprocal_approx_fast(
                                    out=bcr[:], in_=bc_ps[:])
                                nc.vector.tensor_tensor(
                                    ctxT[pair][64 * j:64 * j + 64,
                                               512 * qc:512 * (qc + 1)],
                                    cps_t[h][0:HD, :],
                                    bcr[:], MULT)

                        nc.leave_named_scope(f"attn{qc}", _sid_a, False)
                        for tt in range(4):
                            for c2 in range(2):
                                filler.append(o_group(512 * qc + 128 * tt, c2,
                                                      drain=(qc == NQ - 1)))
                    while filler:
                        pop_filler()

    nc.compile()
    return nc


_NC_CACHE = None


def _get_nc():
    global _NC_CACHE
    if _NC_CACHE is None:
        _NC_CACHE = build_kernel()
    return _NC_CACHE


def make_in_maps(x, Wq, bq, Wk, bk, Wv, bv, Wo, bo):
    in_maps = []
    for c in range(8):
        b, g = c // 4, c % 4
        sl = slice(256 * g, 256 * (g + 1))
        bqg = np.ascontiguousarray(bq[sl].reshape(2, 128).T)
        bkg = np.ascontiguousarray(bk[sl].reshape(2, 128).T)
        bvg = np.ascontiguousarray(np.tile(bv[sl][None, :], (128, 1)))
        in_maps.append({
            "xT": np.ascontiguousarray(x[b].T).astype(BFNP),
            "wq": np.ascontiguousarray(
                Wq[:, sl].reshape(NF, 128, DG).transpose(1, 0, 2)
                .reshape(128, NF * DG)).astype(BFNP),
            "wk": np.ascontiguousarray(
                Wk[:, sl].reshape(NF, 128, DG).transpose(1, 0, 2)
                .reshape(128, NF * DG)).astype(BFNP),
            "wv": np.ascontiguousarray(
                Wv[:, sl].reshape(NF, 128, DG).transpose(1, 0, 2)
                .reshape(128, NF * DG)).astype(BFNP),
            "wo": np.ascontiguousarray(Wo[sl, :].reshape(2, 128, C)).astype(BFNP),
            "bq": bqg.astype(np.float32),
            "bk": bkg.astype(np.float32),
            "bv": bvg.astype(np.float32),
        })
    return in_maps


def combine_outputs(results, bo):
    out = np.empty((2, T, C), np.float32)
    for b in range(2):
        acc = results[4 * b]["out"].astype(np.float32).copy()
        for g in range(1, 4):
            acc += results[4 * b + g]["out"]
        out[b] = acc + bo[None, :]
    return out


def kernel(**inputs):
    from concourse.bass_utils import run_bass_kernel_spmd
    args = {k: np.asarray(v, np.float32) for k, v in inputs.items()}
    nc = _get_nc()
    in_maps = make_in_maps(
        args["x"], args["Wq"], args["bq"], args["Wk"], args["bk"],
        args["Wv"], args["bv"], args["Wo"], args["bo"])
    res = run_bass_kernel_spmd(nc, in_maps, core_ids=list(range(8)))
    return combine_outputs(res.results, args["bo"])


# revision 3
# speedup vs baseline: 1.1884x; 1.1884x over previous
"""Multi-head causal self-attention on 8 trn2 NeuronCores, v2.

Problem: x[2,2048,1024], 16 heads x 64 dim, causal softmax attention,
QKV/O projections with biases.

Sharding: core c handles batch b=c//4, head group g=c%4 (heads 4g..4g+3).
Each core computes its 4 heads' attention plus the partial O-projection;
the host sums the 4 partials per batch and adds bo.

v2 design (vs baseline):
- all matmuls bf16 (same PE rate as f32r, half the DMA/SBUF traffic)
- causal trimming: diagonal k-tiles only compute S / exp q-columns
  >= 128*r (the valid trapezoid); PV runs full-width over a p-tile
  whose left region is memset to zero (keeps PSUM start/stop sane)
- 2-k-tile batches in the attention loop (4 S matmuls, 2 exps, 4 PV
  matmuls) for longer same-shape PE runs; 64-contraction S matmuls
  stream at 2 cols/cycle
- post-exp multiplicative triangle mask on the Pool engine (p is SBUF
  bf16, keeping the DVE out of the S->exp critical chain)
- O-projection packs 2 heads per matmul (128-row contraction)
- softmax denominator: ones-column in V (row 64 of cps); bf16 row cast,
  1-row ones matmul broadcasts it across 64 partitions in PSUM, then
  reciprocal + scale on DVE (reciprocal_approx_fast must start at
  partition 0, and GPSIMD cannot touch PSUM)
- V-projection and O-projection matmuls are interleaved into the
  attention k-loops as PE filler; input DMAs ordered/queued so the
  first projection matmul starts as early as possible
"""
import os
import sys

if os.path.isdir("/opt/trn_rl_repo"):
    sys.path.insert(0, "/opt/trn_rl_repo")

import numpy as np
import ml_dtypes

import concourse.bass as bass  # noqa: F401
import concourse.tile as tile
from concourse import bacc
from concourse import mybir

F32 = mybir.dt.float32
F32R = mybir.dt.float32r
BF16 = mybir.dt.bfloat16
AF = mybir.ActivationFunctionType
ADD = mybir.AluOpType.add
MULT = mybir.AluOpType.mult

T = 2048          # sequence length
C = 1024          # model dim
HG = 4            # heads per core
HD = 64           # head dim
DG = HG * HD      # 256, projected dims per core
NF = C // 128     # 8 feature chunks
NT = T // 128     # 16 token tiles
NQ = T // 512     # 4 q-chunks
SCALE = 0.125     # 1/sqrt(64)
LAG = 4           # exp -> PV pipeline lag (in k-tiles)

BFNP = ml_dtypes.bfloat16


def build_kernel():
    nc = bacc.Bacc("TRN2")
    xT_d = nc.dram_tensor("xT", [C, T], BF16, kind="ExternalInput").ap()
    wq_d = nc.dram_tensor("wq", [128, NF * DG], BF16, kind="ExternalInput").ap()
    wk_d = nc.dram_tensor("wk", [128, NF * DG], BF16, kind="ExternalInput").ap()
    wv_d = nc.dram_tensor("wv", [128, NF * DG], BF16, kind="ExternalInput").ap()
    wo_d = nc.dram_tensor("wo", [2, 128, C], BF16, kind="ExternalInput").ap()
    bq_d = nc.dram_tensor("bq", [128, 2], F32, kind="ExternalInput").ap()
    bk_d = nc.dram_tensor("bk", [128, 2], F32, kind="ExternalInput").ap()
    bv_d = nc.dram_tensor("bv", [128, DG], F32, kind="ExternalInput").ap()
    out_d = nc.dram_tensor("out", [T, C], F32, kind="ExternalOutput").ap()

    with tile.TileContext(nc) as tc:
        with tc.tile_pool(name="persist", bufs=1) as pp:
            qt = pp.tile([128, 2, T], BF16, name="qt")    # [d'128, pair, t]
            kt = pp.tile([128, 2, T], BF16, name="kt")
            vsb = pp.tile([128, NT, HG, HD + 1], BF16, name="vsb")  # [V|1]
            ctxT = [pp.tile([128, T], BF16, name=f"ctxT{p}") for p in range(2)]
            wo_sb = pp.tile([128, 2, C], BF16, name="wo_sb")
            bq_sb = pp.tile([128, 2], F32, name="bq_sb")
            bk_sb = pp.tile([128, 2], F32, name="bk_sb")
            bv_sb = pp.tile([128, DG], F32, name="bv_sb")
            mask2 = pp.tile([128, 2, 128], BF16, name="mask2")
            ones = pp.tile([128, 64], BF16, name="ones")
            nc.gpsimd.memset(ones[:], 1.0)

            # ones column of V_ext
            nc.gpsimd.memset(vsb[:, :, :, HD:HD + 1], 1.0)
            # multiplicative triangle mask: 1 where col >= partition, else 0
            nc.gpsimd.memset(mask2[:], 1.0)
            for j in range(2):
                nc.gpsimd.affine_select(
                    out=mask2[:, j, :],
                    in_=mask2[:, j, :],
                    compare_op=mybir.AluOpType.is_ge,
                    fill=0.0,
                    base=0,
                    pattern=[[1, 128]],
                    channel_multiplier=-1,
                )

            with tc.tile_pool(name="xtp", bufs=1) as xtp, \
                 tc.tile_pool(name="wp", bufs=2) as wp:
                xt = xtp.tile([128, NF, T], BF16, name="xt")

                w_srcs = {"q": wq_d, "k": wk_d, "v": wv_d}
                w_tiles = {}

                def load_w(which, eng, split=False):
                    w_tiles[which] = wp.tile([128, NF, DG], BF16,
                                             name=f"w{which}", tag="w")
                    src_v = w_srcs[which].rearrange("p (f d) -> p f d", f=NF)
                    if split:
                        for f in range(NF):
                            eng.dma_start(w_tiles[which][:, f, :],
                                          src_v[:, f, :])
                    else:
                        eng.dma_start(w_tiles[which][:], src_v)

                load_w("k", nc.scalar, split=True)
                load_w("q", nc.gpsimd)
                load_w("v", nc.gpsimd)
                # first chunk in 512-col pieces so the first matmul can
                # start as soon as 0.125MB lands
                for t4 in range(NQ):
                    nc.sync.dma_start(
                        xt[:, 0, 512 * t4:512 * (t4 + 1)],
                        xT_d[0:128, 512 * t4:512 * (t4 + 1)])
                for f in range(1, NF):
                    nc.sync.dma_start(xt[:, f, :], xT_d[128 * f:128 * (f + 1), :])
                # late, off the critical path: biases + wo (sync queue,
                # after the xt chunks that gate the first matmuls)
                nc.sync.dma_start(bq_sb[:], bq_d)
                nc.sync.dma_start(bk_sb[:], bk_d)
                nc.sync.dma_start(bv_sb[:], bv_d)
                for p in range(2):
                    nc.sync.dma_start(wo_sb[:, p, :], wo_d[p])

                # ---- QK projections: psum [128, 2048] per (dst, dc) ----
                _sid_p, _ = nc.enter_named_scope("proj", False)
                with tc.tile_pool(name="pjp", bufs=2, space="PSUM") as pjp:
                    for dc in range(2):
                        for dst, wkey, b_sb in ((kt, "k", bk_sb),
                                                (qt, "q", bq_sb)):
                            w_sb = w_tiles[wkey]
                            ps = pjp.tile([128, T], F32, name="pjqk", tag="pj")
                            for f in range(NF):
                                lhsT = w_sb[:, f, 128 * dc:128 * (dc + 1)]
                                for t4 in range(NQ):
                                    nc.tensor.matmul(
                                        ps[:, 512 * t4:512 * (t4 + 1)],
                                        lhsT,
                                        xt[:, f, 512 * t4:512 * (t4 + 1)],
                                        start=(f == 0), stop=(f == NF - 1),
                                    )
                            if wkey == "k":
                                # ACT is idle during proj: bias-add + copy
                                nc.scalar.activation(
                                    dst[:, dc, :], ps[:], AF.Identity,
                                    bias=b_sb[:, dc:dc + 1])
                            else:
                                for t4 in range(NQ):
                                    nc.vector.tensor_scalar_add(
                                        dst[:, dc, 512 * t4:512 * (t4 + 1)],
                                        ps[:, 512 * t4:512 * (t4 + 1)],
                                        b_sb[:, dc:dc + 1])
                nc.leave_named_scope("proj", _sid_p, False)

                # ---- phase B + V-proj/O-proj as PE filler ----
                wv_sb = w_tiles["v"]
                with tc.tile_pool(name="pp2", bufs=8) as pbuf, \
                     tc.tile_pool(name="outp", bufs=6) as outp, \
                     tc.tile_pool(name="dnp", bufs=4) as dnp, \
                     tc.tile_pool(name="bcp", bufs=2) as bcp, \
                     tc.tile_pool(name="sps", bufs=2, space="PSUM") as sps, \
                     tc.tile_pool(name="cps", bufs=2, space="PSUM") as cps, \
                     tc.tile_pool(name="vop", bufs=2, space="PSUM") as vop:

                    filler = []

                    def v_group(t):
                        def emit():
                            ps = vop.tile([128, DG], F32, name="vps", tag="vo")
                            for f in range(NF):
                                nc.tensor.matmul(
                                    ps[:],
                                    xt[:, f, 128 * t:128 * (t + 1)],
                                    wv_sb[:, f, :],
                                    start=(f == 0), stop=(f == NF - 1),
                                )
                            nc.vector.tensor_tensor(
                                vsb[:, t, :, 0:HD],
                                ps[:].rearrange("p (h d) -> p h d", h=HG),
                                bv_sb[:].rearrange("p (h d) -> p h d", h=HG),
                                ADD)
                        return emit

                    def o_group(t0, c2, alt=[0], drain=False):
                        def emit():
                            o_ps = vop.tile([128, 512], F32, name="ops",
                                            tag="vo")
                            for p in range(2):
                                nc.tensor.matmul(
                                    o_ps[:],
                                    ctxT[p][:, t0:t0 + 128],
                                    wo_sb[:, p, 512 * c2:512 * (c2 + 1)],
                                    start=(p == 0), stop=(p == 1),
                                )
                            o_sb = outp.tile([128, 512], F32, name="osb",
                                             tag="osb")
                            a = alt[0] = alt[0] + 1
                            if drain:
                                # split copy DVE/ACT + DMAs on two queues so
                                # the end-of-kernel drain parallelizes
                                nc.vector.tensor_copy(o_sb[:, 0:256],
                                                      o_ps[:, 0:256])
                                nc.scalar.activation(o_sb[:, 256:512],
                                                     o_ps[:, 256:512],
                                                     AF.Identity)
                                nc.sync.dma_start(
                                    out_d[t0:t0 + 128,
                                          512 * c2:512 * c2 + 256],
                                    o_sb[:, 0:256])
                                nc.gpsimd.dma_start(
                                    out_d[t0:t0 + 128,
                                          512 * c2 + 256:512 * (c2 + 1)],
                                    o_sb[:, 256:512])
                            else:
                                nc.vector.tensor_copy(o_sb[:], o_ps[:])
                                (nc.sync, nc.gpsimd, nc.scalar)[a % 3].dma_start(
                                    out_d[t0:t0 + 128,
                                          512 * c2:512 * (c2 + 1)],
                                    o_sb[:])
                        return emit

                    for t in range(NT):
                        filler.append(v_group(t))

                    def pop_filler(n=1):
                        for _ in range(n):
                            if filler:
                                filler.pop(0)()

                    for qc in range(NQ):
                        _sid_a, _ = nc.enter_named_scope(f"attn{qc}", False)
                        nkt = 4 * qc + 4
                        for pair in range(2):
                            heads = (2 * pair, 2 * pair + 1)
                            cps_t = {h: cps.tile([HD + 1, 512], F32,
                                                 name=f"cps{h}", tag="cps")
                                     for h in heads}
                            pts = {}
                            # 2-ki batches: 4 S matmuls, 2 exps, then 4 PV
                            # matmuls — longer same-shape PE runs
                            for kb in range(0, nkt + LAG, 2):
                                if kb < nkt:
                                    sts = {}
                                    for ki in (kb, kb + 1):
                                        r = ki - 4 * qc
                                        lo = 128 * r if r > 0 else 0
                                        sts[ki] = (lo, sps.tile(
                                            [128, 2, 512], F32,
                                            name="s_ps", tag="s"))
                                        for j in range(2):
                                            nc.tensor.matmul(
                                                sts[ki][1][:, j, lo:512],
                                                kt[64 * j:64 * j + 64, pair,
                                                   128 * ki:128 * (ki + 1)],
                                                qt[64 * j:64 * j + 64, pair,
                                                   512 * qc + lo:
                                                   512 * (qc + 1)],
                                                start=True, stop=True)
                                    for ki in (kb, kb + 1):
                                        lo, s_ps = sts[ki]
                                        r = ki - 4 * qc
                                        p_t = pbuf.tile([128, 2, 512], BF16,
                                                        name="p", tag="p")
                                        if lo > 0:
                                            # PV runs full-width; zero the
                                            # left-of-trapezoid region
                                            nc.gpsimd.memset(
                                                p_t[:, :, 0:lo], 0.0)
                                        nc.scalar.activation(
                                            p_t[:, :, lo:512],
                                            s_ps[:, :, lo:512],
                                            AF.Exp, scale=SCALE)
                                        if r >= 0:
                                            # zero the above-diag triangle
                                            # (p is SBUF bf16 -> Pool-able)
                                            nc.gpsimd.tensor_tensor(
                                                p_t[:, :, lo:lo + 128],
                                                p_t[:, :, lo:lo + 128],
                                                mask2[:], MULT)
                                        pts[ki] = p_t
                                pop_filler()
                                if kb >= LAG:
                                    for k in (kb - LAG, kb - LAG + 1):
                                        pk = pts.pop(k)
                                        for j, h in enumerate(heads):
                                            nc.tensor.matmul(
                                                cps_t[h][:],
                                                vsb[:, k, h, :],
                                                pk[:, j, :],
                                                start=(k == 0),
                                                stop=(k == nkt - 1),
                                            )
                            # denominators + normalize into ctxT[pair]:
                            # bf16-cast the cps tile (for its denom row),
                            # broadcast the row across 64 partitions with a
                            # 1-row bf16 matmul, reciprocal (partition-0
                            # aligned), then scale ctx
                            for j, h in enumerate(heads):
                                dnb = dnp.tile([65, 512], BF16, name="dnb",
                                               tag="dnb")
                                nc.vector.tensor_copy(dnb[:], cps_t[h][:])
                                bc_ps = vop.tile([64, 512], F32, name="bc",
                                                 tag="vo")
                                nc.tensor.matmul(
                                    bc_ps[:],
                                    ones[64:65, :],
                                    dnb[64:65, :],
                                    start=True, stop=True)
                                bcr = dnp.tile([64, 512], F32, name="bcr",
                                               tag="bcr")
                                nc.vector.reciprocal_approx_fast(
                                    out=bcr[:], in_=bc_ps[:])
                                nc.vector.tensor_tensor(
                                    ctxT[pair][64 * j:64 * j + 64,
                                               512 * qc:512 * (qc + 1)],
                                    cps_t[h][0:HD, :],
                                    bcr[:], MULT)

                        nc.leave_named_scope(f"attn{qc}", _sid_a, False)
                        for tt in range(4):
                            for c2 in range(2):
                                filler.append(o_group(512 * qc + 128 * tt, c2,
                                                      drain=(qc == NQ - 1)))
                    while filler:
                        pop_filler()

    nc.compile()
    return nc


_NC_CACHE = None


def _get_nc():
    global _NC_CACHE
    if _NC_CACHE is None:
        _NC_CACHE = build_kernel()
    return _NC_CACHE


def make_in_maps(x, Wq, bq, Wk, bk, Wv, bv, Wo, bo):
    in_maps = []
    for c in range(8):
        b, g = c // 4, c % 4
        sl = slice(256 * g, 256 * (g + 1))
        bqg = np.ascontiguousarray(bq[sl].reshape(2, 128).T)
        bkg = np.ascontiguousarray(bk[sl].reshape(2, 128).T)
        bvg = np.ascontiguousarray(np.tile(bv[sl][None, :], (128, 1)))
        in_maps.append({
            "xT": np.ascontiguousarray(x[b].T).astype(BFNP),
            "wq": np.ascontiguousarray(
                Wq[:, sl].reshape(NF, 128, DG).transpose(1, 0, 2)
                .reshape(128, NF * DG)).astype(BFNP),
            "wk": np.ascontiguousarray(
                Wk[:, sl].reshape(NF, 128, DG).transpose(1, 0, 2)
                .reshape(128, NF * DG)).astype(BFNP),
            "wv": np.ascontiguousarray(
                Wv[:, sl].reshape(NF, 128, DG).transpose(1, 0, 2)
                .reshape(128, NF * DG)).astype(BFNP),
            "wo": np.ascontiguousarray(Wo[sl, :].reshape(2, 128, C)).astype(BFNP),
            "bq": bqg.astype(np.float32),
            "bk": bkg.astype(np.float32),
            "bv": bvg.astype(np.float32),
        })
    return in_maps


def combine_outputs(results, bo):
    out = np.empty((2, T, C), np.float32)
    for b in range(2):
        acc = results[4 * b]["out"].astype(np.float32).copy()
        for g in range(1, 4):
            acc += results[4 * b + g]["out"]
        out[b] = acc + bo[None, :]
    return out


def kernel(**inputs):
    from concourse.bass_utils import run_bass_kernel_spmd
    args = {k: np.asarray(v, np.float32) for k, v in inputs.items()}
    nc = _get_nc()
    in_maps = make_in_maps(
        args["x"], args["Wq"], args["bq"], args["Wk"], args["bk"],
        args["Wv"], args["bv"], args["Wo"], args["bo"])
    res = run_bass_kernel_spmd(nc, in_maps, core_ids=list(range(8)))
    return combine_outputs(res.results, args["bo"])


# revision 5
# speedup vs baseline: 1.2075x; 1.0160x over previous
"""Multi-head causal self-attention on 8 trn2 NeuronCores, v2.

Problem: x[2,2048,1024], 16 heads x 64 dim, causal softmax attention,
QKV/O projections with biases.

Sharding: core c handles batch b=c//4, head group g=c%4 (heads 4g..4g+3).
Each core computes its 4 heads' attention plus the partial O-projection;
the host sums the 4 partials per batch and adds bo.

v2 design (vs baseline):
- all matmuls bf16 (same PE rate as f32r, half the DMA/SBUF traffic)
- causal trimming: diagonal k-tiles only compute S / exp q-columns
  >= 128*r (the valid trapezoid); PV runs full-width over a p-tile
  whose left region is memset to zero (keeps PSUM start/stop sane)
- 2-k-tile batches in the attention loop (4 S matmuls, 2 exps, 4 PV
  matmuls) for longer same-shape PE runs; 64-contraction S matmuls
  stream at 2 cols/cycle
- post-exp multiplicative triangle mask on the Pool engine (p is SBUF
  bf16, keeping the DVE out of the S->exp critical chain)
- O-projection packs 2 heads per matmul (128-row contraction)
- softmax denominator: ones-column in V (row 64 of cps); bf16 row cast,
  1-row ones matmul broadcasts it across 64 partitions in PSUM, then
  reciprocal + scale on DVE (reciprocal_approx_fast must start at
  partition 0, and GPSIMD cannot touch PSUM)
- V-projection and O-projection matmuls are interleaved into the
  attention k-loops as PE filler; input DMAs ordered/queued so the
  first projection matmul starts as early as possible
"""
import os
import sys

if os.path.isdir("/opt/trn_rl_repo"):
    sys.path.insert(0, "/opt/trn_rl_repo")

import numpy as np
import ml_dtypes

import concourse.bass as bass  # noqa: F401
import concourse.tile as tile
from concourse import bacc
from concourse import mybir

F32 = mybir.dt.float32
F32R = mybir.dt.float32r
BF16 = mybir.dt.bfloat16
AF = mybir.ActivationFunctionType
ADD = mybir.AluOpType.add
MULT = mybir.AluOpType.mult

T = 2048          # sequence length
C = 1024          # model dim
HG = 4            # heads per core
HD = 64           # head dim
DG = HG * HD      # 256, projected dims per core
NF = C // 128     # 8 feature chunks
NT = T // 128     # 16 token tiles
NQ = T // 512     # 4 q-chunks
SCALE = 0.125     # 1/sqrt(64)
LAG = 4           # exp -> PV pipeline lag (in k-tiles)

BFNP = ml_dtypes.bfloat16


def build_kernel():
    nc = bacc.Bacc("TRN2")
    xT_d = nc.dram_tensor("xT", [C, T], BF16, kind="ExternalInput").ap()
    wq_d = nc.dram_tensor("wq", [128, NF * DG], BF16, kind="ExternalInput").ap()
    wk_d = nc.dram_tensor("wk", [128, NF * DG], BF16, kind="ExternalInput").ap()
    wv_d = nc.dram_tensor("wv", [128, NF * DG], BF16, kind="ExternalInput").ap()
    wo_d = nc.dram_tensor("wo", [2, 128, C], BF16, kind="ExternalInput").ap()
    bq_d = nc.dram_tensor("bq", [128, 2], F32, kind="ExternalInput").ap()
    bk_d = nc.dram_tensor("bk", [128, 2], F32, kind="ExternalInput").ap()
    bv_d = nc.dram_tensor("bv", [128, DG], F32, kind="ExternalInput").ap()
    out_d = nc.dram_tensor("out", [T, C], BF16, kind="ExternalOutput").ap()

    with tile.TileContext(nc) as tc:
        with tc.tile_pool(name="persist", bufs=1) as pp:
            qt = pp.tile([128, 2, T], BF16, name="qt")    # [d'128, pair, t]
            kt = pp.tile([128, 2, T], BF16, name="kt")
            vsb = pp.tile([128, NT, HG, HD + 1], BF16, name="vsb")  # [V|1]
            ctxT = [pp.tile([128, T], BF16, name=f"ctxT{p}") for p in range(2)]
            wo_sb = pp.tile([128, 2, C], BF16, name="wo_sb")
            bq_sb = pp.tile([128, 2], F32, name="bq_sb")
            bk_sb = pp.tile([128, 2], F32, name="bk_sb")
            bv_sb = pp.tile([128, DG], F32, name="bv_sb")
            mask2 = pp.tile([128, 2, 128], BF16, name="mask2")
            ones = pp.tile([128, 64], BF16, name="ones")
            nc.gpsimd.memset(ones[:], 1.0)

            # ones column of V_ext
            nc.gpsimd.memset(vsb[:, :, :, HD:HD + 1], 1.0)
            # multiplicative triangle mask: 1 where col >= partition, else 0
            nc.gpsimd.memset(mask2[:], 1.0)
            for j in range(2):
                nc.gpsimd.affine_select(
                    out=mask2[:, j, :],
                    in_=mask2[:, j, :],
                    compare_op=mybir.AluOpType.is_ge,
                    fill=0.0,
                    base=0,
                    pattern=[[1, 128]],
                    channel_multiplier=-1,
                )

            with tc.tile_pool(name="xtp", bufs=1) as xtp, \
                 tc.tile_pool(name="wp", bufs=2) as wp:
                xt = xtp.tile([128, NF, T], BF16, name="xt")

                w_srcs = {"q": wq_d, "k": wk_d, "v": wv_d}
                w_tiles = {}

                def load_w(which, eng, split=False):
                    w_tiles[which] = wp.tile([128, NF, DG], BF16,
                                             name=f"w{which}", tag="w")
                    src_v = w_srcs[which].rearrange("p (f d) -> p f d", f=NF)
                    if split:
                        for f in range(NF):
                            eng.dma_start(w_tiles[which][:, f, :],
                                          src_v[:, f, :])
                    else:
                        eng.dma_start(w_tiles[which][:], src_v)

                load_w("k", nc.scalar, split=True)
                load_w("q", nc.gpsimd)
                load_w("v", nc.gpsimd)
                # first chunk in 512-col pieces so the first matmul can
                # start as soon as 0.125MB lands
                for t4 in range(NQ):
                    nc.sync.dma_start(
                        xt[:, 0, 512 * t4:512 * (t4 + 1)],
                        xT_d[0:128, 512 * t4:512 * (t4 + 1)])
                for f in range(1, NF):
                    nc.sync.dma_start(xt[:, f, :], xT_d[128 * f:128 * (f + 1), :])
                # late, off the critical path: biases + wo (sync queue,
                # after the xt chunks that gate the first matmuls)
                nc.sync.dma_start(bq_sb[:], bq_d)
                nc.sync.dma_start(bk_sb[:], bk_d)
                nc.sync.dma_start(bv_sb[:], bv_d)
                for p in range(2):
                    nc.sync.dma_start(wo_sb[:, p, :], wo_d[p])

                # ---- QK projections: psum [128, 2048] per (dst, dc) ----
                _sid_p, _ = nc.enter_named_scope("proj", False)
                with tc.tile_pool(name="pjp", bufs=2, space="PSUM") as pjp:
                    for dc in range(2):
                        for dst, wkey, b_sb in ((kt, "k", bk_sb),
                                                (qt, "q", bq_sb)):
                            w_sb = w_tiles[wkey]
                            ps = pjp.tile([128, T], F32, name="pjqk", tag="pj")
                            for f in range(NF):
                                lhsT = w_sb[:, f, 128 * dc:128 * (dc + 1)]
                                for t4 in range(NQ):
                                    nc.tensor.matmul(
                                        ps[:, 512 * t4:512 * (t4 + 1)],
                                        lhsT,
                                        xt[:, f, 512 * t4:512 * (t4 + 1)],
                                        start=(f == 0), stop=(f == NF - 1),
                                    )
                            if wkey == "k":
                                # ACT is idle during proj: bias-add + copy
                                nc.scalar.activation(
                                    dst[:, dc, :], ps[:], AF.Identity,
                                    bias=b_sb[:, dc:dc + 1])
                            else:
                                for t4 in range(NQ):
                                    nc.vector.tensor_scalar_add(
                                        dst[:, dc, 512 * t4:512 * (t4 + 1)],
                                        ps[:, 512 * t4:512 * (t4 + 1)],
                                        b_sb[:, dc:dc + 1])
                nc.leave_named_scope("proj", _sid_p, False)

                # ---- phase B + V-proj/O-proj as PE filler ----
                wv_sb = w_tiles["v"]
                with tc.tile_pool(name="pp2", bufs=8) as pbuf, \
                     tc.tile_pool(name="opp", bufs=8) as opp, \
                     tc.tile_pool(name="outp", bufs=6) as outp, \
                     tc.tile_pool(name="dnp", bufs=4) as dnp, \
                     tc.tile_pool(name="bcp", bufs=2) as bcp, \
                     tc.tile_pool(name="sps", bufs=2, space="PSUM") as sps, \
                     tc.tile_pool(name="cps", bufs=2, space="PSUM") as cps, \
                     tc.tile_pool(name="vop", bufs=2, space="PSUM") as vop:

                    filler = []

                    def v_group(t):
                        def emit():
                            ps = vop.tile([128, DG], F32, name="vps", tag="vo")
                            for f in range(NF):
                                nc.tensor.matmul(
                                    ps[:],
                                    xt[:, f, 128 * t:128 * (t + 1)],
                                    wv_sb[:, f, :],
                                    start=(f == 0), stop=(f == NF - 1),
                                )
                            nc.vector.tensor_tensor(
                                vsb[:, t, :, 0:HD],
                                ps[:].rearrange("p (h d) -> p h d", h=HG),
                                bv_sb[:].rearrange("p (h d) -> p h d", h=HG),
                                ADD)
                        return emit

                    def o_stage_a(t0, c2, store):
                        def emit():
                            o_ps = vop.tile([128, 512], F32, name="opsA",
                                            tag="vo")
                            nc.tensor.matmul(
                                o_ps[:], ctxT[0][:, t0:t0 + 128],
                                wo_sb[:, 0, 512 * c2:512 * (c2 + 1)],
                                start=True, stop=True)
                            part = opp.tile([128, 512], BF16, name="opart",
                                            tag="op")
                            nc.vector.tensor_copy(part[:], o_ps[:])
                            store[(t0, c2)] = part
                        return emit

                    def o_stage_b(t0, c2, store, alt=[0]):
                        def emit():
                            o_ps = vop.tile([128, 512], F32, name="opsB",
                                            tag="vo")
                            nc.tensor.matmul(
                                o_ps[:], ctxT[1][:, t0:t0 + 128],
                                wo_sb[:, 1, 512 * c2:512 * (c2 + 1)],
                                start=True, stop=True)
                            o_sb = outp.tile([128, 512], BF16, name="osbB",
                                             tag="osb")
                            nc.vector.tensor_tensor(
                                o_sb[:], o_ps[:], store[(t0, c2)], ADD)
                            a = alt[0] = alt[0] + 1
                            (nc.sync, nc.gpsimd, nc.scalar)[a % 3].dma_start(
                                out_d[t0:t0 + 128, 512 * c2:512 * (c2 + 1)],
                                o_sb[:])
                        return emit

                    o_parts = {}

                    def o_group(t0, c2, alt=[0], drain=False):
                        def emit():
                            o_ps = vop.tile([128, 512], F32, name="ops",
                                            tag="vo")
                            for p in range(2):
                                nc.tensor.matmul(
                                    o_ps[:],
                                    ctxT[p][:, t0:t0 + 128],
                                    wo_sb[:, p, 512 * c2:512 * (c2 + 1)],
                                    start=(p == 0), stop=(p == 1),
                                )
                            o_sb = outp.tile([128, 512], BF16, name="osb",
                                             tag="osb")
                            a = alt[0] = alt[0] + 1
                            if drain:
                                # split copy DVE/ACT + DMAs on two queues so
                                # the end-of-kernel drain parallelizes
                                nc.vector.tensor_copy(o_sb[:, 0:256],
                                                      o_ps[:, 0:256])
                                nc.scalar.activation(o_sb[:, 256:512],
                                                     o_ps[:, 256:512],
                                                     AF.Identity)
                                nc.sync.dma_start(
                                    out_d[t0:t0 + 128,
                                          512 * c2:512 * c2 + 256],
                                    o_sb[:, 0:256])
                                nc.gpsimd.dma_start(
                                    out_d[t0:t0 + 128,
                                          512 * c2 + 256:512 * (c2 + 1)],
                                    o_sb[:, 256:512])
                            else:
                                nc.vector.tensor_copy(o_sb[:], o_ps[:])
                                (nc.sync, nc.gpsimd, nc.scalar)[a % 3].dma_start(
                                    out_d[t0:t0 + 128,
                                          512 * c2:512 * (c2 + 1)],
                                    o_sb[:])
                        return emit

                    for t in range(NT):
                        filler.append(v_group(t))

                    def pop_filler(n=1):
                        for _ in range(n):
                            if filler:
                                filler.pop(0)()

                    for qc in range(NQ):
                        _sid_a, _ = nc.enter_named_scope(f"attn{qc}", False)
                        nkt = 4 * qc + 4
                        for pair in range(2):
                            heads = (2 * pair, 2 * pair + 1)
                            cps_t = {h: cps.tile([HD + 1, 512], F32,
                                                 name=f"cps{h}", tag="cps")
                                     for h in heads}
                            pts = {}
                            # 2-ki batches: 4 S matmuls, 2 exps, then 4 PV
                            # matmuls — longer same-shape PE runs
                            for kb in range(0, nkt + LAG, 2):
                                if kb < nkt:
                                    sts = {}
                                    for ki in (kb, kb + 1):
                                        r = ki - 4 * qc
                                        lo = 128 * r if r > 0 else 0
                                        sts[ki] = (lo, sps.tile(
                                            [128, 2, 512], F32,
                                            name="s_ps", tag="s"))
                                        for j in range(2):
                                            nc.tensor.matmul(
                                                sts[ki][1][:, j, lo:512],
                                                kt[64 * j:64 * j + 64, pair,
                                                   128 * ki:128 * (ki + 1)],
                                                qt[64 * j:64 * j + 64, pair,
                                                   512 * qc + lo:
                                                   512 * (qc + 1)],
                                                start=True, stop=True)
                                    for ki in (kb, kb + 1):
                                        lo, s_ps = sts[ki]
                                        r = ki - 4 * qc
                                        p_t = pbuf.tile([128, 2, 512], BF16,
                                                        name="p", tag="p")
                                        if lo > 0:
                                            # PV runs full-width; zero the
                                            # left-of-trapezoid region
                                            nc.gpsimd.memset(
                                                p_t[:, :, 0:lo], 0.0)
                                        nc.scalar.activation(
                                            p_t[:, :, lo:512],
                                            s_ps[:, :, lo:512],
                                            AF.Exp, scale=SCALE)
                                        if r >= 0:
                                            # zero the above-diag triangle
                                            # (p is SBUF bf16 -> Pool-able)
                                            nc.gpsimd.tensor_tensor(
                                                p_t[:, :, lo:lo + 128],
                                                p_t[:, :, lo:lo + 128],
                                                mask2[:], MULT)
                                        pts[ki] = p_t
                                pop_filler()
                                if kb >= LAG:
                                    for k in (kb - LAG, kb - LAG + 1):
                                        pk = pts.pop(k)
                                        for j, h in enumerate(heads):
                                            nc.tensor.matmul(
                                                cps_t[h][:],
                                                vsb[:, k, h, :],
                                                pk[:, j, :],
                                                start=(k == 0),
                                                stop=(k == nkt - 1),
                                            )
                            # denominators + normalize into ctxT[pair]:
                            # bf16-cast the cps tile (for its denom row),
                            # broadcast the row across 64 partitions with a
                            # 1-row bf16 matmul, reciprocal (partition-0
                            # aligned), then scale ctx
                            if qc == NQ - 1 and pair == 0:
                                for tt in range(4):
                                    for c2 in range(2):
                                        filler.append(o_stage_a(
                                            512 * qc + 128 * tt, c2, o_parts))
                            for j, h in enumerate(heads):
                                dnb = dnp.tile([65, 512], BF16, name="dnb",
                                               tag="dnb")
                                nc.vector.tensor_copy(dnb[:], cps_t[h][:])
                                bc_ps = vop.tile([64, 512], F32, name="bc",
                                                 tag="vo")
                                nc.tensor.matmul(
                                    bc_ps[:],
                                    ones[64:65, :],
                                    dnb[64:65, :],
                                    start=True, stop=True)
                                bcr = dnp.tile([64, 512], F32, name="bcr",
                                               tag="bcr")
                                nc.vector.reciprocal_approx_fast(
                                    out=bcr[:], in_=bc_ps[:])
                                nc.vector.tensor_tensor(
                                    ctxT[pair][64 * j:64 * j + 64,
                                               512 * qc:512 * (qc + 1)],
                                    cps_t[h][0:HD, :],
                                    bcr[:], MULT)

                        nc.leave_named_scope(f"attn{qc}", _sid_a, False)
                        for tt in range(4):
                            for c2 in range(2):
                                if qc == NQ - 1:
                                    filler.append(o_stage_b(
                                        512 * qc + 128 * tt, c2, o_parts))
                                else:
                                    filler.append(o_group(
                                        512 * qc + 128 * tt, c2))
                    while filler:
                        pop_filler()

    nc.compile()
    return nc


_NC_CACHE = None


def _get_nc():
    global _NC_CACHE
    if _NC_CACHE is None:
        _NC_CACHE = build_kernel()
    return _NC_CACHE


def make_in_maps(x, Wq, bq, Wk, bk, Wv, bv, Wo, bo):
    in_maps = []
    for c in range(8):
        b, g = c // 4, c % 4
        sl = slice(256 * g, 256 * (g + 1))
        bqg = np.ascontiguousarray(bq[sl].reshape(2, 128).T)
        bkg = np.ascontiguousarray(bk[sl].reshape(2, 128).T)
        bvg = np.ascontiguousarray(np.tile(bv[sl][None, :], (128, 1)))
        in_maps.append({
            "xT": np.ascontiguousarray(x[b].T).astype(BFNP),
            "wq": np.ascontiguousarray(
                Wq[:, sl].reshape(NF, 128, DG).transpose(1, 0, 2)
                .reshape(128, NF * DG)).astype(BFNP),
            "wk": np.ascontiguousarray(
                Wk[:, sl].reshape(NF, 128, DG).transpose(1, 0, 2)
                .reshape(128, NF * DG)).astype(BFNP),
            "wv": np.ascontiguousarray(
                Wv[:, sl].reshape(NF, 128, DG).transpose(1, 0, 2)
                .reshape(128, NF * DG)).astype(BFNP),
            "wo": np.ascontiguousarray(Wo[sl, :].reshape(2, 128, C)).astype(BFNP),
            "bq": bqg.astype(np.float32),
            "bk": bkg.astype(np.float32),
            "bv": bvg.astype(np.float32),
        })
    return in_maps


def combine_outputs(results, bo):
    out = np.empty((2, T, C), np.float32)
    for b in range(2):
        acc = results[4 * b]["out"].astype(np.float32).copy()
        for g in range(1, 4):
            acc += results[4 * b + g]["out"]
        out[b] = acc + bo[None, :]
    return out


def kernel(**inputs):
    from concourse.bass_utils import run_bass_kernel_spmd
    args = {k: np.asarray(v, np.float32) for k, v in inputs.items()}
    nc = _get_nc()
    in_maps = make_in_maps(
        args["x"], args["Wq"], args["bq"], args["Wk"], args["bk"],
        args["Wv"], args["bv"], args["Wo"], args["bo"])
    res = run_bass_kernel_spmd(nc, in_maps, core_ids=list(range(8)))
    return combine_outputs(res.results, args["bo"])


# revision 6
# speedup vs baseline: 1.2223x; 1.0123x over previous
"""Multi-head causal self-attention on 8 trn2 NeuronCores, v2.

Problem: x[2,2048,1024], 16 heads x 64 dim, causal softmax attention,
QKV/O projections with biases.

Sharding: core c handles batch b=c//4, head group g=c%4 (heads 4g..4g+3).
Each core computes its 4 heads' attention plus the partial O-projection;
the host sums the 4 partials per batch and adds bo.

v2 design (vs baseline):
- all matmuls bf16 (same PE rate as f32r, half the DMA/SBUF traffic)
- causal trimming: diagonal k-tiles only compute S / exp q-columns
  >= 128*r (the valid trapezoid); PV runs full-width over a p-tile
  whose left region is memset to zero (keeps PSUM start/stop sane)
- 2-k-tile batches in the attention loop (4 S matmuls, 2 exps, 4 PV
  matmuls) for longer same-shape PE runs; 64-contraction S matmuls
  stream at 2 cols/cycle
- post-exp multiplicative triangle mask on the Pool engine (p is SBUF
  bf16, keeping the DVE out of the S->exp critical chain)
- O-projection packs 2 heads per matmul (128-row contraction)
- softmax denominator: ones-column in V (row 64 of cps); bf16 row cast,
  1-row ones matmul broadcasts it across 64 partitions in PSUM, then
  reciprocal + scale on DVE (reciprocal_approx_fast must start at
  partition 0, and GPSIMD cannot touch PSUM)
- V-projection and O-projection matmuls are interleaved into the
  attention k-loops as PE filler; input DMAs ordered/queued so the
  first projection matmul starts as early as possible
"""
import os
import sys

if os.path.isdir("/opt/trn_rl_repo"):
    sys.path.insert(0, "/opt/trn_rl_repo")

import numpy as np
import ml_dtypes

import concourse.bass as bass  # noqa: F401
import concourse.tile as tile
from concourse import bacc
from concourse import mybir

F32 = mybir.dt.float32
F32R = mybir.dt.float32r
BF16 = mybir.dt.bfloat16
AF = mybir.ActivationFunctionType
ADD = mybir.AluOpType.add
MULT = mybir.AluOpType.mult

T = 2048          # sequence length
C = 1024          # model dim
HG = 4            # heads per core
HD = 64           # head dim
DG = HG * HD      # 256, projected dims per core
NF = C // 128     # 8 feature chunks
NT = T // 128     # 16 token tiles
NQ = T // 512     # 4 q-chunks
SCALE = 0.125     # 1/sqrt(64)
LAG = 4           # exp -> PV pipeline lag (in k-tiles)

BFNP = ml_dtypes.bfloat16


def build_kernel():
    nc = bacc.Bacc("TRN2")
    xT_d = nc.dram_tensor("xT", [C, T], BF16, kind="ExternalInput").ap()
    wq_d = nc.dram_tensor("wq", [128, NF * DG], BF16, kind="ExternalInput").ap()
    wk_d = nc.dram_tensor("wk", [128, NF * DG], BF16, kind="ExternalInput").ap()
    wv_d = nc.dram_tensor("wv", [128, NF * DG], BF16, kind="ExternalInput").ap()
    wo_d = nc.dram_tensor("wo", [2, 128, C], BF16, kind="ExternalInput").ap()
    bq_d = nc.dram_tensor("bq", [128, 2], F32, kind="ExternalInput").ap()
    bk_d = nc.dram_tensor("bk", [128, 2], F32, kind="ExternalInput").ap()
    bv_d = nc.dram_tensor("bv", [128, DG], F32, kind="ExternalInput").ap()
    out_d = nc.dram_tensor("out", [T, C], BF16, kind="ExternalOutput").ap()

    with tile.TileContext(nc) as tc:
        with tc.tile_pool(name="persist", bufs=1) as pp:
            qt = pp.tile([128, 2, T], BF16, name="qt")    # [d'128, pair, t]
            kt = pp.tile([128, 2, T], BF16, name="kt")
            vsb = pp.tile([128, NT, HG, HD + 1], BF16, name="vsb")  # [V|1]
            ctxT = [pp.tile([128, T], BF16, name=f"ctxT{p}") for p in range(2)]
            wo_sb = pp.tile([128, 2, C], BF16, name="wo_sb")
            bq_sb = pp.tile([128, 2], F32, name="bq_sb")
            bk_sb = pp.tile([128, 2], F32, name="bk_sb")
            bv_sb = pp.tile([128, DG], F32, name="bv_sb")
            mask2 = pp.tile([128, 2, 128], BF16, name="mask2")
            ones = pp.tile([128, 64], BF16, name="ones")
            nc.gpsimd.memset(ones[:], 1.0)

            # ones column of V_ext
            nc.gpsimd.memset(vsb[:, :, :, HD:HD + 1], 1.0)
            # multiplicative triangle mask: 1 where col >= partition, else 0
            nc.gpsimd.memset(mask2[:], 1.0)
            for j in range(2):
                nc.gpsimd.affine_select(
                    out=mask2[:, j, :],
                    in_=mask2[:, j, :],
                    compare_op=mybir.AluOpType.is_ge,
                    fill=0.0,
                    base=0,
                    pattern=[[1, 128]],
                    channel_multiplier=-1,
                )

            with tc.tile_pool(name="xtp", bufs=1) as xtp, \
                 tc.tile_pool(name="wp", bufs=2) as wp:
                xt = xtp.tile([128, NF, T], BF16, name="xt")

                w_srcs = {"q": wq_d, "k": wk_d, "v": wv_d}
                w_tiles = {}

                def load_w(which, eng, split=False):
                    w_tiles[which] = wp.tile([128, NF, DG], BF16,
                                             name=f"w{which}", tag="w")
                    src_v = w_srcs[which].rearrange("p (f d) -> p f d", f=NF)
                    if split:
                        # f0 alone (64KB, gates the first matmul), then the
                        # rest as one contiguous 3.5KB-per-partition transfer
                        eng.dma_start(w_tiles[which][:, 0, :], src_v[:, 0, :])
                        eng.dma_start(w_tiles[which][:, 1:NF, :],
                                      src_v[:, 1:NF, :])
                    else:
                        eng.dma_start(w_tiles[which][:], src_v)

                load_w("k", nc.scalar, split=True)
                load_w("q", nc.gpsimd)
                load_w("v", nc.gpsimd)
                # first chunk in 512-col pieces so the first matmul can
                # start as soon as 0.125MB lands
                for t4 in range(NQ):
                    nc.sync.dma_start(
                        xt[:, 0, 512 * t4:512 * (t4 + 1)],
                        xT_d[0:128, 512 * t4:512 * (t4 + 1)])
                for f in range(1, NF):
                    nc.sync.dma_start(xt[:, f, :], xT_d[128 * f:128 * (f + 1), :])
                # late, off the critical path: biases + wo (sync queue,
                # after the xt chunks that gate the first matmuls)
                nc.sync.dma_start(bq_sb[:], bq_d)
                nc.sync.dma_start(bk_sb[:], bk_d)
                nc.sync.dma_start(bv_sb[:], bv_d)
                for p in range(2):
                    nc.sync.dma_start(wo_sb[:, p, :], wo_d[p])

                # ---- QK projections: psum [128, 2048] per (dst, dc) ----
                _sid_p, _ = nc.enter_named_scope("proj", False)
                with tc.tile_pool(name="pjp", bufs=2, space="PSUM") as pjp:
                    for dc in range(2):
                        for dst, wkey, b_sb in ((kt, "k", bk_sb),
                                                (qt, "q", bq_sb)):
                            w_sb = w_tiles[wkey]
                            ps = pjp.tile([128, T], F32, name="pjqk", tag="pj")
                            for f in range(NF):
                                lhsT = w_sb[:, f, 128 * dc:128 * (dc + 1)]
                                for t4 in range(NQ):
                                    nc.tensor.matmul(
                                        ps[:, 512 * t4:512 * (t4 + 1)],
                                        lhsT,
                                        xt[:, f, 512 * t4:512 * (t4 + 1)],
                                        start=(f == 0), stop=(f == NF - 1),
                                    )
                            if wkey == "k":
                                # ACT is idle during proj: bias-add + copy
                                nc.scalar.activation(
                                    dst[:, dc, :], ps[:], AF.Identity,
                                    bias=b_sb[:, dc:dc + 1])
                            else:
                                for t4 in range(NQ):
                                    nc.vector.tensor_scalar_add(
                                        dst[:, dc, 512 * t4:512 * (t4 + 1)],
                                        ps[:, 512 * t4:512 * (t4 + 1)],
                                        b_sb[:, dc:dc + 1])
                nc.leave_named_scope("proj", _sid_p, False)

                # ---- phase B + V-proj/O-proj as PE filler ----
                wv_sb = w_tiles["v"]
                with tc.tile_pool(name="pp2", bufs=8) as pbuf, \
                     tc.tile_pool(name="opp", bufs=8) as opp, \
                     tc.tile_pool(name="outp", bufs=6) as outp, \
                     tc.tile_pool(name="dnp", bufs=4) as dnp, \
                     tc.tile_pool(name="bcp", bufs=2) as bcp, \
                     tc.tile_pool(name="sps", bufs=2, space="PSUM") as sps, \
                     tc.tile_pool(name="cps", bufs=2, space="PSUM") as cps, \
                     tc.tile_pool(name="vop", bufs=2, space="PSUM") as vop:

                    filler = []

                    def v_group(t):
                        def emit():
                            ps = vop.tile([128, DG], F32, name="vps", tag="vo")
                            for f in range(NF):
                                nc.tensor.matmul(
                                    ps[:],
                                    xt[:, f, 128 * t:128 * (t + 1)],
                                    wv_sb[:, f, :],
                                    start=(f == 0), stop=(f == NF - 1),
                                )
                            nc.vector.tensor_tensor(
                                vsb[:, t, :, 0:HD],
                                ps[:].rearrange("p (h d) -> p h d", h=HG),
                                bv_sb[:].rearrange("p (h d) -> p h d", h=HG),
                                ADD)
                        return emit

                    def o_stage_a(t0, c2, store):
                        def emit():
                            o_ps = vop.tile([128, 512], F32, name="opsA",
                                            tag="vo")
                            nc.tensor.matmul(
                                o_ps[:], ctxT[0][:, t0:t0 + 128],
                                wo_sb[:, 0, 512 * c2:512 * (c2 + 1)],
                                start=True, stop=True)
                            part = opp.tile([128, 512], BF16, name="opart",
                                            tag="op")
                            nc.vector.tensor_copy(part[:], o_ps[:])
                            store[(t0, c2)] = part
                        return emit

                    def o_stage_b(t0, c2, store, alt=[0]):
                        def emit():
                            o_ps = vop.tile([128, 512], F32, name="opsB",
                                            tag="vo")
                            nc.tensor.matmul(
                                o_ps[:], ctxT[1][:, t0:t0 + 128],
                                wo_sb[:, 1, 512 * c2:512 * (c2 + 1)],
                                start=True, stop=True)
                            o_sb = outp.tile([128, 512], BF16, name="osbB",
                                             tag="osb")
                            nc.vector.tensor_tensor(
                                o_sb[:], o_ps[:], store[(t0, c2)], ADD)
                            a = alt[0] = alt[0] + 1
                            (nc.sync, nc.gpsimd, nc.scalar)[a % 3].dma_start(
                                out_d[t0:t0 + 128, 512 * c2:512 * (c2 + 1)],
                                o_sb[:])
                        return emit

                    o_parts = {}

                    def o_group(t0, c2, alt=[0], drain=False):
                        def emit():
                            o_ps = vop.tile([128, 512], F32, name="ops",
                                            tag="vo")
                            for p in range(2):
                                nc.tensor.matmul(
                                    o_ps[:],
                                    ctxT[p][:, t0:t0 + 128],
                                    wo_sb[:, p, 512 * c2:512 * (c2 + 1)],
                                    start=(p == 0), stop=(p == 1),
                                )
                            o_sb = outp.tile([128, 512], BF16, name="osb",
                                             tag="osb")
                            a = alt[0] = alt[0] + 1
                            if drain:
                                # split copy DVE/ACT + DMAs on two queues so
                                # the end-of-kernel drain parallelizes
                                nc.vector.tensor_copy(o_sb[:, 0:256],
                                                      o_ps[:, 0:256])
                                nc.scalar.activation(o_sb[:, 256:512],
                                                     o_ps[:, 256:512],
                                                     AF.Identity)
                                nc.sync.dma_start(
                                    out_d[t0:t0 + 128,
                                          512 * c2:512 * c2 + 256],
                                    o_sb[:, 0:256])
                                nc.gpsimd.dma_start(
                                    out_d[t0:t0 + 128,
                                          512 * c2 + 256:512 * (c2 + 1)],
                                    o_sb[:, 256:512])
                            else:
                                nc.vector.tensor_copy(o_sb[:], o_ps[:])
                                (nc.sync, nc.gpsimd, nc.scalar)[a % 3].dma_start(
                                    out_d[t0:t0 + 128,
                                          512 * c2:512 * (c2 + 1)],
                                    o_sb[:])
                        return emit

                    for t in range(NT):
                        filler.append(v_group(t))

                    def pop_filler(n=1):
                        for _ in range(n):
                            if filler:
                                filler.pop(0)()

                    for qc in range(NQ):
                        _sid_a, _ = nc.enter_named_scope(f"attn{qc}", False)
                        nkt = 4 * qc + 4
                        for pair in range(2):
                            heads = (2 * pair, 2 * pair + 1)
                            cps_t = {h: cps.tile([HD + 1, 512], F32,
                                                 name=f"cps{h}", tag="cps")
                                     for h in heads}
                            pts = {}
                            # 2-ki batches: 4 S matmuls, 2 exps, then 4 PV
                            # matmuls — longer same-shape PE runs
                            for kb in range(0, nkt + LAG, 2):
                                if kb < nkt:
                                    sts = {}
                                    for ki in (kb, kb + 1):
                                        r = ki - 4 * qc
                                        lo = 128 * r if r > 0 else 0
                                        sts[ki] = (lo, sps.tile(
                                            [128, 2, 512], F32,
                                            name="s_ps", tag="s"))
                                        for j in range(2):
                                            nc.tensor.matmul(
                                                sts[ki][1][:, j, lo:512],
                                                kt[64 * j:64 * j + 64, pair,
                                                   128 * ki:128 * (ki + 1)],
                                                qt[64 * j:64 * j + 64, pair,
                                                   512 * qc + lo:
                                                   512 * (qc + 1)],
                                                start=True, stop=True)
                                    for ki in (kb, kb + 1):
                                        lo, s_ps = sts[ki]
                                        r = ki - 4 * qc
                                        p_t = pbuf.tile([128, 2, 512], BF16,
                                                        name="p", tag="p")
                                        if lo > 0:
                                            # PV runs full-width; zero the
                                            # left-of-trapezoid region
                                            nc.gpsimd.memset(
                                                p_t[:, :, 0:lo], 0.0)
                                        nc.scalar.activation(
                                            p_t[:, :, lo:512],
                                            s_ps[:, :, lo:512],
                                            AF.Exp, scale=SCALE)
                                        if r >= 0:
                                            # zero the above-diag triangle
                                            # (p is SBUF bf16 -> Pool-able)
                                            nc.gpsimd.tensor_tensor(
                                                p_t[:, :, lo:lo + 128],
                                                p_t[:, :, lo:lo + 128],
                                                mask2[:], MULT)
                                        pts[ki] = p_t
                                pop_filler()
                                if kb >= LAG:
                                    for k in (kb - LAG, kb - LAG + 1):
                                        pk = pts.pop(k)
                                        for j, h in enumerate(heads):
                                            nc.tensor.matmul(
                                                cps_t[h][:],
                                                vsb[:, k, h, :],
                                                pk[:, j, :],
                                                start=(k == 0),
                                                stop=(k == nkt - 1),
                                            )
                            # denominators + normalize into ctxT[pair]:
                            # bf16-cast the cps tile (for its denom row),
                            # broadcast the row across 64 partitions with a
                            # 1-row bf16 matmul, reciprocal (partition-0
                            # aligned), then scale ctx
                            if qc == NQ - 1 and pair == 0:
                                for tt in range(4):
                                    for c2 in range(2):
                                        filler.append(o_stage_a(
                                            512 * qc + 128 * tt, c2, o_parts))
                            for j, h in enumerate(heads):
                                dnb = dnp.tile([65, 512], BF16, name="dnb",
                                               tag="dnb")
                                nc.vector.tensor_copy(dnb[:], cps_t[h][:])
                                bc_ps = vop.tile([64, 512], F32, name="bc",
                                                 tag="vo")
                                nc.tensor.matmul(
                                    bc_ps[:],
                                    ones[64:65, :],
                                    dnb[64:65, :],
                                    start=True, stop=True)
                                bcr = dnp.tile([64, 512], F32, name="bcr",
                                               tag="bcr")
                                nc.vector.reciprocal_approx_fast(
                                    out=bcr[:], in_=bc_ps[:])
                                nc.vector.tensor_tensor(
                                    ctxT[pair][64 * j:64 * j + 64,
                                               512 * qc:512 * (qc + 1)],
                                    cps_t[h][0:HD, :],
                                    bcr[:], MULT)

                        nc.leave_named_scope(f"attn{qc}", _sid_a, False)
                        for tt in range(4):
                            for c2 in range(2):
                                if qc == NQ - 1:
                                    filler.append(o_stage_b(
                                        512 * qc + 128 * tt, c2, o_parts))
                                else:
                                    filler.append(o_group(
                                        512 * qc + 128 * tt, c2))
                    while filler:
                        pop_filler()

    nc.compile()
    return nc


_NC_CACHE = None


def _get_nc():
    global _NC_CACHE
    if _NC_CACHE is None:
        _NC_CACHE = build_kernel()
    return _NC_CACHE


def make_in_maps(x, Wq, bq, Wk, bk, Wv, bv, Wo, bo):
    in_maps = []
    for c in range(8):
        b, g = c // 4, c % 4
        sl = slice(256 * g, 256 * (g + 1))
        bqg = np.ascontiguousarray(bq[sl].reshape(2, 128).T)
        bkg = np.ascontiguousarray(bk[sl].reshape(2, 128).T)
        bvg = np.ascontiguousarray(np.tile(bv[sl][None, :], (128, 1)))
        in_maps.append({
            "xT": np.ascontiguousarray(x[b].T).astype(BFNP),
            "wq": np.ascontiguousarray(
                Wq[:, sl].reshape(NF, 128, DG).transpose(1, 0, 2)
                .reshape(128, NF * DG)).astype(BFNP),
            "wk": np.ascontiguousarray(
                Wk[:, sl].reshape(NF, 128, DG).transpose(1, 0, 2)
                .reshape(128, NF * DG)).astype(BFNP),
            "wv": np.ascontiguousarray(
                Wv[:, sl].reshape(NF, 128, DG).transpose(1, 0, 2)
                .reshape(128, NF * DG)).astype(BFNP),
            "wo": np.ascontiguousarray(Wo[sl, :].reshape(2, 128, C)).astype(BFNP),
            "bq": bqg.astype(np.float32),
            "bk": bkg.astype(np.float32),
            "bv": bvg.astype(np.float32),
        })
    return in_maps


def combine_outputs(results, bo):
    out = np.empty((2, T, C), np.float32)
    for b in range(2):
        acc = results[4 * b]["out"].astype(np.float32).copy()
        for g in range(1, 4):
            acc += results[4 * b + g]["out"]
        out[b] = acc + bo[None, :]
    return out


def kernel(**inputs):
    from concourse.bass_utils import run_bass_kernel_spmd
    args = {k: np.asarray(v, np.float32) for k, v in inputs.items()}
    nc = _get_nc()
    in_maps = make_in_maps(
        args["x"], args["Wq"], args["bq"], args["Wk"], args["bk"],
        args["Wv"], args["bv"], args["Wo"], args["bo"])
    res = run_bass_kernel_spmd(nc, in_maps, core_ids=list(range(8)))
    return combine_outputs(res.results, args["bo"])
